# revision 7
# baseline (speedup 1.0000x reference)
"""MultiHeadGATODE (gcn ode, merge=cat) Trainium2 kernel, 8-core SPMD.

Math: for head i, out_i = relu(norm * segsum(((h @ W_i.T + b_i) * norm)[src], dst)).
By linearity:  segsum(((h W + b) * norm)[src]) = aggH @ W.T + s * b
  where aggH[d] = sum_{e: dst_e=d} norm[src_e] * h[src_e]   (aggregate in h-space)
        s[d]    = sum_{e: dst_e=d} norm[src_e]
So no per-head work before aggregation, and no inter-core traffic at all:
h*norm (fp16) is replicated to every core's DRAM; each core owns 1/8 of the
dst nodes (degree-balanced-packed into 128-row blocks; host unpermutes the
output rows at the end) and
  1) dma_gathers the 1KB fp16 hn rows for its edges (one call per
     (block, lo/hi index half), sorted by src for HBM locality),
  2) scatter-adds them into a per-block PSUM bank via one-hot matmuls
     (lhsT = one-hot built on DVE from iota==dst_local; the same one-hot
     also aggregates s via a [128,2] norm-hi/lo lhsT),
  3) transposes aggH on the PE (fp16), applies Wcat (fp16 matmuls) +
     s*bcat + relu*norm_dst (ACT), writes its out shard.

Precision: gathered payload is single fp16 (~5e-4 rel err), phase B in fp16
(~1e-3); comfortably under the 2e-2 gate. (The old hi/lo fp16 split gave
4e-7 but doubled both the gather bytes and the PE one-hot stream.)

Bottleneck (HW-profiled): the SWDGE Q7 descriptor generator runs ~5.35ns/row
serialized on the Pool engine (the Q7 loop is ~120cyc/16 idxs; one core pair
per call, calls serialize at the Pool sequencer), so E/8 = 200K rows/core
puts a ~1.1ms floor on this gather-based design. DMA (2.84ns/row at 1KB)
and PE (~5 cyc/edge) ride well under it.
"""

import os
import sys

for _p in ("/opt/trn_rl_repo",):
    if _p not in sys.path and os.path.isdir(_p):
        sys.path.insert(0, _p)

import numpy as np

from contextlib import ExitStack

import concourse.bass as bass
import concourse.tile as tile
from concourse import bacc, mybir
from concourse.bass_utils import run_bass_kernel_spmd

F16 = mybir.dt.float16
F32 = mybir.dt.float32
I16 = mybir.dt.int16

NCORES = 8
SPLIT = 32768          # int16 gather-index limit
# Trailing -1 gather indices are trimmed by the Q7 DGE (skips their
# descriptors + DMA).
PAD_SKIP = True
GAT_BUFS = 4

TRACE = bool(os.environ.get("KERNEL_TRACE"))


def _ensure_ntff_hook():
    """The agent image's antenv lacks axon_hooks; synthesize it so
    run_bass_kernel_spmd(trace=True) can NTFF-profile via libaxon_pjrt."""
    import types
    import ctypes
    import contextlib

    try:
        from antenv.axon_hooks import get_axon_ntff_profile_hook  # noqa: F401
        return
    except ImportError:
        pass
    so_path = "/opt/axon/libaxon_pjrt.so"
    if not os.path.exists(so_path):
        return
    lib = ctypes.CDLL(so_path)
    if not hasattr(lib, "axon_start_nrt_profile"):
        return
    lib.axon_start_nrt_profile.argtypes = [ctypes.POINTER(ctypes.c_int64), ctypes.c_size_t]
    lib.axon_start_nrt_profile.restype = ctypes.c_int64
    lib.axon_stop_nrt_profile.argtypes = [ctypes.c_char_p]
    lib.axon_stop_nrt_profile.restype = ctypes.c_int64

    @contextlib.contextmanager
    def _hook(output_dir, device_ids):
        import jax

        jax.devices()
        if device_ids:
            ids = (ctypes.c_int64 * len(device_ids))(*device_ids)
            rc = lib.axon_start_nrt_profile(ids, len(device_ids))
        else:
            rc = lib.axon_start_nrt_profile(None, 0)
        if rc != 0:
            raise RuntimeError(f"axon_start_nrt_profile rc={rc}")
        try:
            yield
        finally:
            n = lib.axon_stop_nrt_profile(str(output_dir).encode())
            if n < 0:
                raise RuntimeError(f"axon_stop_nrt_profile rc={n}")

    _hook_obj = _hook

    mod = types.ModuleType("antenv.axon_hooks")
    mod.get_axon_ntff_profile_hook = lambda: _hook_obj
    mod.set_axon_ntff_profile_hook = lambda h: None
    sys.modules["antenv.axon_hooks"] = mod


if TRACE:
    _ensure_ntff_hook()


def _split16(x):
    hi = x.astype(np.float16)
    lo = (x - hi.astype(np.float32)).astype(np.float16)
    return hi, lo


def _prep(h, W, b, norm, src, dst):
    """Host-side staging: sort/group edges, build per-core input maps."""
    N, IN = h.shape
    H, OUT, _ = W.shape
    HOUT = H * OUT
    E = src.shape[0]
    assert N % NCORES == 0
    NSH = N // NCORES
    NBLK = (NSH + 127) // 128

    h = np.asarray(h, np.float32)
    W = np.asarray(W, np.float32)
    b = np.asarray(b, np.float32)
    norm = np.asarray(norm, np.float32).reshape(N)
    src = np.asarray(src, np.int64)
    dst = np.asarray(dst, np.int64)

    # h * norm in single fp16 (rel err ~5e-4, within the 2e-2 gate)
    hn = (h * norm[:, None]).astype(np.float16)  # [N, IN]

    nsrc_hi_all, nsrc_lo_all = _split16(norm)

    hiflag = (src >= SPLIT).astype(np.int64)
    nblocks_tot = NCORES * NBLK

    # --- degree-balanced dst -> (block, pos) packing (per core) ---------
    # The device writes block b of core c to out rows [128b, 128b+bs);
    # the host unpermutes at the end, so block membership is free. Pack
    # dsts so every block sees ~equal lo-half and hi-half edge counts:
    # this minimizes the group-count maxima (padding) that gate the gat
    # tile sizes and PE stream.
    n_lo_d = np.bincount(dst[hiflag == 0], minlength=N).astype(np.int64)
    n_hi_d = np.bincount(dst[hiflag == 1], minlength=N).astype(np.int64)
    blk_of_dst = np.empty(N, np.int64)   # block within core
    pos_of_dst = np.empty(N, np.int64)   # row within block
    cap_full = np.full(NBLK, 128, np.int64)
    if NSH % 128:
        cap_full[NBLK - 1] = NSH % 128
    tgt_lo = max(1.0, n_lo_d.sum() / nblocks_tot)
    tgt_hi = max(1.0, n_hi_d.sum() / nblocks_tot)
    for c in range(NCORES):
        dd = np.arange(c * NSH, (c + 1) * NSH)
        order_d = np.argsort(-(n_lo_d[dd] + n_hi_d[dd]), kind="stable")
        sums_lo = np.zeros(NBLK)
        sums_hi = np.zeros(NBLK)
        cnt = np.zeros(NBLK, np.int64)
        cap = cap_full * 1
        # scale partial-block targets
        scale = cap / 128.0
        for d_local in order_d:
            d = c * NSH + d_local
            load = np.maximum(
                (sums_lo + n_lo_d[d]) / (tgt_lo * scale + 1e-9),
                (sums_hi + n_hi_d[d]) / (tgt_hi * scale + 1e-9),
            )
            load[cnt >= cap] = np.inf
            bbb = int(np.argmin(load))
            blk_of_dst[d] = bbb
            pos_of_dst[d] = cnt[bbb]
            sums_lo[bbb] += n_lo_d[d]
            sums_hi[bbb] += n_hi_d[d]
            cnt[bbb] += 1

    # Sort edges: core-block-major, then half, then src (ascending src gives
    # the HBM gather some locality; aggregation is order-invariant).
    blk_of_edge = (dst // NSH) * NBLK + blk_of_dst[dst]
    order2 = np.lexsort((src, hiflag, blk_of_edge))
    s_src = src[order2]
    s_dst = dst[order2]
    s_hi = hiflag[order2]
    blk_of_edge = blk_of_edge[order2]

    key2 = blk_of_edge * 2 + s_hi
    seg_bounds = np.searchsorted(key2, np.arange(2 * nblocks_tot + 1))
    seg_cnt = np.diff(seg_bounds)  # [2*nblocks_tot] edges per (block, half)

    g_lo = int(np.max([-(-int(c) // 128) for c in seg_cnt[0::2]] or [0]))
    g_hi = int(np.max([-(-int(c) // 128) for c in seg_cnt[1::2]] or [0]))
    g_lo = max(g_lo, 1)
    g_hi = max(g_hi, 1)
    CG = max(g_lo, g_hi)               # one dma_gather call per (block, half)
    NG = NBLK * (g_lo + g_hi)          # groups per core
    S = NG * 128                        # slots per core

    # slot position for every edge
    # slot base of (blk, half): blk*(g_lo+g_hi)*128 + half*g_lo*128 (within core)
    blk_in_core = blk_of_edge % NBLK
    seg_id = blk_of_edge * 2 + s_hi
    rank = np.arange(E) - seg_bounds[seg_id]
    slot_in_core = (blk_in_core * (g_lo + g_hi) + s_hi * g_lo) * 128 + rank
    core_of_edge = blk_of_edge // NBLK
    assert (rank < (np.where(s_hi == 1, g_hi, g_lo)) * 128).all(), "group overflow"

    # per-core slot arrays
    idx16 = np.full((NCORES, S), -1 if PAD_SKIP else 0, np.int16)
    dloc = np.full((NCORES, S), -1.0, np.float16)
    ns_hi = np.zeros((NCORES, S), np.float16)
    ns_lo = np.zeros((NCORES, S), np.float16)

    flat = core_of_edge * S + slot_in_core
    idx16.reshape(-1)[flat] = (s_src - s_hi * SPLIT).astype(np.int16)
    dloc.reshape(-1)[flat] = pos_of_dst[s_dst].astype(np.float16)
    ns_hi.reshape(-1)[flat] = nsrc_hi_all[s_src]
    ns_lo.reshape(-1)[flat] = nsrc_lo_all[s_src]

    # per-(block, half) valid-index counts; one dma_gather call per segment
    counts = []
    for c in range(NCORES):
        row = []
        for bb in range(NBLK):
            for half, g_tot in ((0, g_lo), (1, g_hi)):
                seg = int(seg_cnt[(c * NBLK + bb) * 2 + half])
                if PAD_SKIP:
                    row.append(int(np.clip(seg, 0, g_tot * 128)))
                else:
                    row.append(g_tot * 128)
        counts.append(row)
    counts_dram = np.asarray(counts, np.int32)[:, None, :]  # [NC, 1, ncalls]

    # staging layouts
    idx_dram = idx16.reshape(NCORES, S // 16, 16).transpose(0, 2, 1)  # [NC,16,S/16]
    idx_dram = np.tile(idx_dram, (1, 8, 1)).copy()                    # [NC,128,S/16]
    dloc_dram = dloc.reshape(NCORES, NG, 128).transpose(0, 2, 1).copy()
    nsrc_dram = (
        np.stack([ns_hi.reshape(NCORES, NG, 128), ns_lo.reshape(NCORES, NG, 128)], axis=2)
        .transpose(0, 3, 1, 2)
        .reshape(NCORES, 128, 2 * NG)
        .copy()
    )

    # norm_dst [NC, 128, NBLK] in packed (block, pos) order; also the shard-row
    # permutation for host-side output reassembly: dst d lives at shard row
    # blk_of_dst[d]*128 + pos_of_dst[d] of core d // NSH.
    shard_row_of_dst = (blk_of_dst * 128 + pos_of_dst).astype(np.int64)
    npad = np.ones((NCORES, NBLK * 128), np.float32)
    np.put_along_axis(
        npad,
        shard_row_of_dst.reshape(NCORES, NSH),
        norm.reshape(NCORES, NSH),
        axis=1,
    )
    ndst_dram = npad.reshape(NCORES, NBLK, 128).transpose(0, 2, 1).copy()

    # weights (fp16 for 1 cyc/row PE streams in phase B)
    Wcat = np.concatenate([W[i].T for i in range(H)], axis=1)  # [IN, HOUT]
    assert Wcat.shape == (IN, HOUT)
    nkch = IN // 128
    wcat_dram = Wcat.reshape(nkch, 128, HOUT).astype(np.float16)
    bcat = np.concatenate([b[i] for i in range(H)])            # [HOUT]
    bcat2 = np.tile(bcat[None, :], (2, 1)).astype(np.float16)  # [2, HOUT]

    iota = np.tile(np.arange(128, dtype=np.float16)[None, :], (128, CG)).copy()
    ident = np.eye(128, dtype=np.float32)

    shared = {
        "hn": hn,
        "wcat": wcat_dram,
        "bcat2": bcat2,
        "iota": iota,
        "ident": ident,
    }
    in_maps = []
    for c in range(NCORES):
        m = dict(shared)
        m["idx"] = idx_dram[c]
        m["dloc"] = dloc_dram[c]
        m["nsrc"] = nsrc_dram[c]
        m["ndst"] = ndst_dram[c]
        m["cnts"] = counts_dram[c]
        in_maps.append(m)

    geom = dict(
        N=N, IN=IN, HOUT=HOUT, NSH=NSH, NBLK=NBLK, g_lo=g_lo, g_hi=g_hi,
        CG=CG, NG=NG, S=S, ncalls=counts_dram.shape[2],
    )
    return in_maps, geom, shard_row_of_dst.reshape(NCORES, NSH)


def _build(geom):
    N, IN, HOUT = geom["N"], geom["IN"], geom["HOUT"]
    NSH, NBLK, g_lo, g_hi, CG, NG, S = (
        geom["NSH"], geom["NBLK"], geom["g_lo"], geom["g_hi"], geom["CG"],
        geom["NG"], geom["S"],
    )
    nkch = IN // 128

    nc = bacc.Bacc(
        "TRN2",
        target_bir_lowering=False,
        debug=False,
        num_devices=NCORES,
        num_swdge_queues=4,
    )

    hnd = nc.dram_tensor("hn", [N, IN], F16, kind="ExternalInput").ap()
    idx = nc.dram_tensor("idx", [128, S // 16], I16, kind="ExternalInput").ap()
    dlocd = nc.dram_tensor("dloc", [128, NG], F16, kind="ExternalInput").ap()
    nsrcd = nc.dram_tensor("nsrc", [128, 2 * NG], F16, kind="ExternalInput").ap()
    ndstd = nc.dram_tensor("ndst", [128, NBLK], F32, kind="ExternalInput").ap()
    wcatd = nc.dram_tensor("wcat", [nkch, 128, HOUT], F16, kind="ExternalInput").ap()
    bcat2d = nc.dram_tensor("bcat2", [2, HOUT], F16, kind="ExternalInput").ap()
    iotad = nc.dram_tensor("iota", [128, CG * 128], F16, kind="ExternalInput").ap()
    identd = nc.dram_tensor("ident", [128, 128], F32, kind="ExternalInput").ap()
    cntsd = nc.dram_tensor("cnts", [1, geom["ncalls"]], mybir.dt.int32, kind="ExternalInput").ap()
    out = nc.dram_tensor("out", [NSH, HOUT], F32, kind="ExternalOutput").ap()

    with tile.TileContext(nc) as tc, ExitStack() as ctx:
        consts = ctx.enter_context(tc.tile_pool(name="consts", bufs=1))
        bigs = ctx.enter_context(tc.tile_pool(name="bigs", bufs=1))
        gatp = ctx.enter_context(tc.tile_pool(name="gat", bufs=GAT_BUFS))
        ohp = ctx.enter_context(tc.tile_pool(name="oh", bufs=3))
        sbB = ctx.enter_context(tc.tile_pool(name="sbB", bufs=2))
        psA = ctx.enter_context(tc.tile_pool(name="psA", bufs=2, space="PSUM"))
        psS = ctx.enter_context(tc.tile_pool(name="psS", bufs=2, space="PSUM"))
        psT = ctx.enter_context(tc.tile_pool(name="psT", bufs=2, space="PSUM"))
        psO = ctx.enter_context(tc.tile_pool(name="psO", bufs=2, space="PSUM"))

        # constants
        iota_sb = consts.tile([128, CG * 128], F16, tag="iota")
        nc.sync.dma_start(iota_sb[:], iotad[:])
        ident_sb = consts.tile([128, 128], F32, tag="ident")
        nc.sync.dma_start(ident_sb[:], identd[:])
        bcat2_sb = consts.tile([2, HOUT], F16, tag="bcat2")
        nc.sync.dma_start(bcat2_sb[:], bcat2d[:])
        ndst_sb = consts.tile([128, NBLK], F32, tag="ndst")
        nc.sync.dma_start(ndst_sb[:], ndstd[:])
        wcat_sb = []
        for k in range(nkch):
            w = consts.tile([128, HOUT], F16, tag=f"wcat{k}")
            nc.sync.dma_start(w[:], wcatd[k])
            wcat_sb.append(w)
        idx_sb = bigs.tile([128, S // 16], I16, tag="idx")
        nc.sync.dma_start(idx_sb[:], idx[:])
        dloc_sb = bigs.tile([128, NG], F16, tag="dloc")
        nc.sync.dma_start(dloc_sb[:], dlocd[:])
        nsrc_sb = bigs.tile([128, 2 * NG], F16, tag="nsrc")
        nc.sync.dma_start(nsrc_sb[:], nsrcd[:])
        cnts_sb = bigs.tile([1, geom["ncalls"]], mybir.dt.int32, tag="cnts")
        nc.sync.dma_start(cnts_sb[:], cntsd[:])

        hn_lo_view = hnd[0:min(SPLIT, N), :]
        hn_hi_view = hnd[SPLIT:N, :] if N > SPLIT else None
        qrr = [0]  # gather queue round-robin / call counter

        for bb in range(NBLK):
            rows = min(128, NSH - bb * 128)
            ps_main = psA.tile([128, IN], F32, tag="main")
            ps_s = psS.tile([2, 128], F32, tag="s")
            goff = bb * (g_lo + g_hi)  # group offset of this block
            n_emit = g_lo + (g_hi if hn_hi_view is not None else 0)
            gi = 0  # groups emitted so far for this block
            for half, (g_tot, base) in enumerate(
                [(g_lo, hn_lo_view), (g_hi, hn_hi_view)]
            ):
                if base is None:
                    continue
                gcall = goff + (g_lo if half else 0)
                ng = g_tot
                icol0 = gcall * 8  # 128 idx / 16 per group
                gat = gatp.tile([128, CG, IN], F16, tag="gat")
                if qrr[0] < GAT_BUFS:
                    # First rotation of the pool: zero the physical
                    # buffers so skipped pad slots hold finite values
                    # (0 * garbage-NaN would poison the PSUM).
                    nc.vector.memset(gat[:], 0.0)
                if PAD_SKIP:
                    nreg = nc.alloc_registers(engines=(mybir.EngineType.Pool,))
                    nc.gpsimd.reg_load(nreg, cnts_sb[0:1, qrr[0] : qrr[0] + 1])
                    cnt = nreg
                else:
                    cnt = ng * 128
                nc.gpsimd.dma_gather(
                    gat[:, :ng, :],
                    base,
                    idx_sb[:, icol0 : icol0 + ng * 8],
                    ng * 128,
                    cnt,
                    IN,
                    queue_num=qrr[0] % 4,
                )
                qrr[0] += 1
                oh = ohp.tile([128, CG * 128], F16, tag="oh")
                dl = (
                    dloc_sb[:, gcall : gcall + ng]
                    .unsqueeze(2)
                    .broadcast_to([128, ng, 128])
                )
                nc.vector.tensor_tensor(
                    oh[:, : ng * 128],
                    iota_sb[:, : ng * 128],
                    dl,
                    mybir.AluOpType.is_equal,
                )
                for g in range(ng):
                    lhs = oh[:, g * 128 : (g + 1) * 128]
                    gg = gcall + g
                    first = gi == 0
                    last = gi == n_emit - 1
                    nc.tensor.matmul(
                        ps_main[:], lhs, gat[:, g, :], start=first, stop=last
                    )
                    nc.tensor.matmul(
                        ps_s[:],
                        nsrc_sb[:, 2 * gg : 2 * gg + 2],
                        lhs,
                        start=first,
                        stop=last,
                    )
                    gi += 1
            assert gi == n_emit

            # phase B (fp32 transpose, fp16 Wcat streams: 1 cyc/row on PE)
            aggH = sbB.tile([128, IN], F32, tag="aggH")
            nc.scalar.copy(aggH[:], ps_main[:])
            s_sb = sbB.tile([2, 128], F16, tag="s_sb")
            nc.vector.tensor_copy(s_sb[:], ps_s[:])
            aggHT = sbB.tile([128, IN], F16, tag="aggHT")
            for k in range(nkch):
                ps_t = psT.tile([128, 128], F32, tag="pt")
                nc.tensor.transpose(ps_t[:], aggH[:, k * 128 : (k + 1) * 128], ident_sb[:])
                nc.vector.tensor_copy(aggHT[:, k * 128 : (k + 1) * 128], ps_t[:])
            ps_o = psO.tile([128, HOUT], F32, tag="po")
            for k in range(nkch):
                nc.tensor.matmul(
                    ps_o[:],
                    aggHT[:, k * 128 : (k + 1) * 128],
                    wcat_sb[k][:],
                    start=(k == 0),
                    stop=False,
                )
            nc.tensor.matmul(ps_o[:], s_sb[:], bcat2_sb[:], start=False, stop=True)
            outsb = sbB.tile([128, HOUT], F32, tag="outsb")
            nc.scalar.activation(
                outsb[:],
                ps_o[:],
                mybir.ActivationFunctionType.Relu,
                scale=ndst_sb[:, bb : bb + 1],
            )
            nc.sync.dma_start(out[bb * 128 : bb * 128 + rows, :], outsb[:rows, :])

    nc.compile()
    return nc


_CACHE = {}


def kernel(h, W, b, norm, src, dst):
    h = np.asarray(h)
    in_maps, geom, shard_rows = _prep(h, W, b, norm, src, dst)
    key = tuple(sorted(geom.items()))
    if key not in _CACHE:
        _CACHE[key] = _build(geom)
    nc = _CACHE[key]
    res = run_bass_kernel_spmd(
        nc, in_maps, list(range(NCORES)), trace=TRACE
    )
    shards = [
        res.results[c]["out"][shard_rows[c]] for c in range(NCORES)
    ]
    out = np.concatenate(shards, axis=0).astype(np.float32)
    if TRACE and res.exec_time_ns is not None:
        print(f"HW exec time: {res.exec_time_ns} ns")
    kernel._last = res
    return out


# revision 15
# speedup vs baseline: 1.8387x; 1.8387x over previous
"""MultiHeadGATODE (gcn ode, merge=cat) Trainium2 kernel, 8-core SPMD.

Math: for head i, out_i = relu(norm * segsum(((h @ W_i.T + b_i) * norm)[src], dst)).
By linearity:  segsum(((h W + b) * norm)[src]) = aggH @ W.T + s * b
  where aggH[d] = sum_{e: dst_e=d} norm[src_e] * h[src_e]   (aggregate in h-space)
        s[d]    = sum_{e: dst_e=d} norm[src_e]
So no per-head work before aggregation, and no inter-core traffic at all:
h*norm (fp16) is replicated to every core's DRAM; each core owns 1/8 of the
dst nodes (degree-balanced-packed into 128-row blocks; host unpermutes the
output rows at the end) and
  1) dma_gathers the 1KB fp16 hn rows for its edges (one call per
     (block, lo/hi index half), sorted by src for HBM locality),
  2) scatter-adds them into a per-block PSUM bank via one-hot matmuls
     (lhsT = one-hot built on DVE from iota==dst_local; the same one-hot
     also aggregates s via a [128,2] norm-hi/lo lhsT),
  3) transposes aggH on the PE (fp16), applies Wcat (fp16 matmuls) +
     s*bcat + relu*norm_dst (ACT), writes its out shard.

Precision: gathered payload is single fp16 (~5e-4 rel err), phase B in fp16
(~1e-3); comfortably under the 2e-2 gate. (The old hi/lo fp16 split gave
4e-7 but doubled both the gather bytes and the PE one-hot stream.)

Bottleneck (HW-profiled): the SWDGE Q7 descriptor generator runs ~5.35ns/row
serialized on the Pool engine (the Q7 loop is ~120cyc/16 idxs; one core pair
per call, calls serialize at the Pool sequencer), so E/8 = 200K rows/core
puts a ~1.1ms floor on this gather-based design. DMA (2.84ns/row at 1KB)
and PE (~5 cyc/edge) ride well under it.
"""

import os
import sys

for _p in ("/opt/trn_rl_repo",):
    if _p not in sys.path and os.path.isdir(_p):
        sys.path.insert(0, _p)

import numpy as np

from contextlib import ExitStack

import concourse.bass as bass
import concourse.tile as tile
from concourse import bacc, mybir
from concourse.bass_utils import run_bass_kernel_spmd

F16 = mybir.dt.float16
F32 = mybir.dt.float32
I16 = mybir.dt.int16

NCORES = 8
SPLIT = 32768          # int16 gather-index limit
CG = 8                 # gather groups per dma_gather call
# Trailing -1 gather indices are trimmed by the Q7 DGE (skips their
# descriptors + DMA).
PAD_SKIP = True
GAT_BUFS = 7

TRACE = bool(os.environ.get("KERNEL_TRACE"))


def _ensure_ntff_hook():
    """The agent image's antenv lacks axon_hooks; synthesize it so
    run_bass_kernel_spmd(trace=True) can NTFF-profile via libaxon_pjrt."""
    import types
    import ctypes
    import contextlib

    try:
        from antenv.axon_hooks import get_axon_ntff_profile_hook  # noqa: F401
        return
    except ImportError:
        pass
    so_path = "/opt/axon/libaxon_pjrt.so"
    if not os.path.exists(so_path):
        return
    lib = ctypes.CDLL(so_path)
    if not hasattr(lib, "axon_start_nrt_profile"):
        return
    lib.axon_start_nrt_profile.argtypes = [ctypes.POINTER(ctypes.c_int64), ctypes.c_size_t]
    lib.axon_start_nrt_profile.restype = ctypes.c_int64
    lib.axon_stop_nrt_profile.argtypes = [ctypes.c_char_p]
    lib.axon_stop_nrt_profile.restype = ctypes.c_int64

    @contextlib.contextmanager
    def _hook(output_dir, device_ids):
        import jax

        jax.devices()
        if device_ids:
            ids = (ctypes.c_int64 * len(device_ids))(*device_ids)
            rc = lib.axon_start_nrt_profile(ids, len(device_ids))
        else:
            rc = lib.axon_start_nrt_profile(None, 0)
        if rc != 0:
            raise RuntimeError(f"axon_start_nrt_profile rc={rc}")
        try:
            yield
        finally:
            n = lib.axon_stop_nrt_profile(str(output_dir).encode())
            if n < 0:
                raise RuntimeError(f"axon_stop_nrt_profile rc={n}")

    _hook_obj = _hook

    mod = types.ModuleType("antenv.axon_hooks")
    mod.get_axon_ntff_profile_hook = lambda: _hook_obj
    mod.set_axon_ntff_profile_hook = lambda h: None
    sys.modules["antenv.axon_hooks"] = mod


if TRACE:
    _ensure_ntff_hook()


def _split16(x):
    hi = x.astype(np.float16)
    lo = (x - hi.astype(np.float32)).astype(np.float16)
    return hi, lo


def _prep(h, W, b, norm, src, dst):
    """Host-side staging: sort/group edges, build per-core input maps."""
    N, IN = h.shape
    H, OUT, _ = W.shape
    HOUT = H * OUT
    E = src.shape[0]
    assert N % NCORES == 0
    NSH = N // NCORES
    NBLK = (NSH + 127) // 128

    h = np.asarray(h, np.float32)
    W = np.asarray(W, np.float32)
    b = np.asarray(b, np.float32)
    norm = np.asarray(norm, np.float32).reshape(N)
    src = np.asarray(src, np.int64)
    dst = np.asarray(dst, np.int64)

    # h * norm in single fp16 (rel err ~5e-4, within the 2e-2 gate)
    hn = (h * norm[:, None]).astype(np.float16)  # [N, IN]

    nsrc_hi_all, nsrc_lo_all = _split16(norm)

    hiflag = (src >= SPLIT).astype(np.int64)
    nblocks_tot = NCORES * NBLK

    # --- degree-balanced dst -> (block, pos) packing (per core) ---------
    # The device writes block b of core c to out rows [128b, 128b+bs);
    # the host unpermutes at the end, so block membership is free. Pack
    # dsts so every block sees ~equal lo-half and hi-half edge counts:
    # this minimizes the group-count maxima (padding) that gate the gat
    # tile sizes and PE stream.
    n_lo_d = np.bincount(dst[hiflag == 0], minlength=N).astype(np.int64)
    n_hi_d = np.bincount(dst[hiflag == 1], minlength=N).astype(np.int64)
    blk_of_dst = np.empty(N, np.int64)   # block within core
    pos_of_dst = np.empty(N, np.int64)   # row within block
    cap_full = np.full(NBLK, 128, np.int64)
    if NSH % 128:
        cap_full[NBLK - 1] = NSH % 128
    tgt_lo = max(1.0, n_lo_d.sum() / nblocks_tot)
    tgt_hi = max(1.0, n_hi_d.sum() / nblocks_tot)
    for c in range(NCORES):
        dd = np.arange(c * NSH, (c + 1) * NSH)
        order_d = np.argsort(-(n_lo_d[dd] + n_hi_d[dd]), kind="stable")
        sums_lo = np.zeros(NBLK)
        sums_hi = np.zeros(NBLK)
        cnt = np.zeros(NBLK, np.int64)
        cap = cap_full * 1
        # scale partial-block targets
        scale = cap / 128.0
        for d_local in order_d:
            d = c * NSH + d_local
            load = np.maximum(
                (sums_lo + n_lo_d[d]) / (tgt_lo * scale + 1e-9),
                (sums_hi + n_hi_d[d]) / (tgt_hi * scale + 1e-9),
            )
            load[cnt >= cap] = np.inf
            bbb = int(np.argmin(load))
            blk_of_dst[d] = bbb
            pos_of_dst[d] = cnt[bbb]
            sums_lo[bbb] += n_lo_d[d]
            sums_hi[bbb] += n_hi_d[d]
            cnt[bbb] += 1

    # Sort edges: core-block-major, then half, then src (ascending src gives
    # the HBM gather some locality; aggregation is order-invariant).
    blk_of_edge = (dst // NSH) * NBLK + blk_of_dst[dst]
    order2 = np.lexsort((src, hiflag, blk_of_edge))
    s_src = src[order2]
    s_dst = dst[order2]
    s_hi = hiflag[order2]
    blk_of_edge = blk_of_edge[order2]

    key2 = blk_of_edge * 2 + s_hi
    seg_bounds = np.searchsorted(key2, np.arange(2 * nblocks_tot + 1))
    seg_cnt = np.diff(seg_bounds)  # [2*nblocks_tot] edges per (block, half)

    g_lo = int(np.max([-(-int(c) // 128) for c in seg_cnt[0::2]] or [0]))
    g_hi = int(np.max([-(-int(c) // 128) for c in seg_cnt[1::2]] or [0]))
    g_lo = max(g_lo, 1)
    g_hi = max(g_hi, 1)
    NG = NBLK * (g_lo + g_hi)          # groups per core
    S = NG * 128                        # slots per core

    # slot position for every edge
    # slot base of (blk, half): blk*(g_lo+g_hi)*128 + half*g_lo*128 (within core)
    blk_in_core = blk_of_edge % NBLK
    seg_id = blk_of_edge * 2 + s_hi
    rank = np.arange(E) - seg_bounds[seg_id]
    slot_in_core = (blk_in_core * (g_lo + g_hi) + s_hi * g_lo) * 128 + rank
    core_of_edge = blk_of_edge // NBLK
    assert (rank < (np.where(s_hi == 1, g_hi, g_lo)) * 128).all(), "group overflow"

    # per-core slot arrays
    idx16 = np.full((NCORES, S), -1 if PAD_SKIP else 0, np.int16)
    dloc = np.full((NCORES, S), -1.0, np.float16)
    ns_hi = np.zeros((NCORES, S), np.float16)
    ns_lo = np.zeros((NCORES, S), np.float16)

    flat = core_of_edge * S + slot_in_core
    idx16.reshape(-1)[flat] = (s_src - s_hi * SPLIT).astype(np.int16)
    dloc.reshape(-1)[flat] = pos_of_dst[s_dst].astype(np.float16)
    ns_hi.reshape(-1)[flat] = nsrc_hi_all[s_src]
    ns_lo.reshape(-1)[flat] = nsrc_lo_all[s_src]

    # per-(block, half, call) valid-index counts, call plan must match _build
    def calls_of(g):
        full, rem = divmod(g, CG)
        return [CG] * full + ([rem] if rem else [])

    counts = []
    for c in range(NCORES):
        row = []
        for bb in range(NBLK):
            for half, g_tot in ((0, g_lo), (1, g_hi)):
                seg = int(seg_cnt[(c * NBLK + bb) * 2 + half])
                coff = 0
                for ng in calls_of(g_tot):
                    if PAD_SKIP:
                        row.append(int(np.clip(seg - coff * 128, 0, ng * 128)))
                    else:
                        row.append(ng * 128)
                    coff += ng
        counts.append(row)
    counts_dram = np.asarray(counts, np.int32)[:, None, :]  # [NC, 1, ncalls]

    # staging layouts
    idx_dram = idx16.reshape(NCORES, S // 16, 16).transpose(0, 2, 1)  # [NC,16,S/16]
    idx_dram = np.tile(idx_dram, (1, 8, 1)).copy()                    # [NC,128,S/16]
    dloc_dram = dloc.reshape(NCORES, NG, 128).transpose(0, 2, 1).copy()
    nsrc_dram = (
        np.stack([ns_hi.reshape(NCORES, NG, 128), ns_lo.reshape(NCORES, NG, 128)], axis=2)
        .transpose(0, 3, 1, 2)
        .reshape(NCORES, 128, 2 * NG)
        .copy()
    )

    # norm_dst [NC, 128, NBLK] in packed (block, pos) order; also the shard-row
    # permutation for host-side output reassembly: dst d lives at shard row
    # blk_of_dst[d]*128 + pos_of_dst[d] of core d // NSH.
    shard_row_of_dst = (blk_of_dst * 128 + pos_of_dst).astype(np.int64)
    npad = np.ones((NCORES, NBLK * 128), np.float32)
    np.put_along_axis(
        npad,
        shard_row_of_dst.reshape(NCORES, NSH),
        norm.reshape(NCORES, NSH),
        axis=1,
    )
    ndst_dram = npad.reshape(NCORES, NBLK, 128).transpose(0, 2, 1).copy()

    # weights (fp16 for 1 cyc/row PE streams in phase B)
    Wcat = np.concatenate([W[i].T for i in range(H)], axis=1)  # [IN, HOUT]
    assert Wcat.shape == (IN, HOUT)
    nkch = IN // 128
    wcat_dram = Wcat.reshape(nkch, 128, HOUT).astype(np.float16)
    bcat = np.concatenate([b[i] for i in range(H)])            # [HOUT]
    bcat2 = np.tile(bcat[None, :], (2, 1)).astype(np.float16)  # [2, HOUT]

    iota = np.tile(np.arange(128, dtype=np.float16)[None, :], (128, CG)).copy()
    ident = np.eye(128, dtype=np.float32)

    shared = {
        "hn": hn,
        "wcat": wcat_dram,
        "bcat2": bcat2,
        "iota": iota,
        "ident": ident,
    }
    in_maps = []
    for c in range(NCORES):
        m = dict(shared)
        m["idx"] = idx_dram[c]
        m["dloc"] = dloc_dram[c]
        m["nsrc"] = nsrc_dram[c]
        m["ndst"] = ndst_dram[c]
        m["cnts"] = counts_dram[c]
        in_maps.append(m)

    geom = dict(
        N=N, IN=IN, HOUT=HOUT, NSH=NSH, NBLK=NBLK, g_lo=g_lo, g_hi=g_hi,
        NG=NG, S=S, ncalls=counts_dram.shape[2],
    )
    return in_maps, geom, shard_row_of_dst.reshape(NCORES, NSH)


def _build(geom):
    N, IN, HOUT = geom["N"], geom["IN"], geom["HOUT"]
    NSH, NBLK, g_lo, g_hi, NG, S = (
        geom["NSH"], geom["NBLK"], geom["g_lo"], geom["g_hi"],
        geom["NG"], geom["S"],
    )
    nkch = IN // 128

    nc = bacc.Bacc(
        "TRN2",
        target_bir_lowering=False,
        debug=False,
        num_devices=NCORES,
        num_swdge_queues=4,
    )

    hnd = nc.dram_tensor("hn", [N, IN], F16, kind="ExternalInput").ap()
    idx = nc.dram_tensor("idx", [128, S // 16], I16, kind="ExternalInput").ap()
    dlocd = nc.dram_tensor("dloc", [128, NG], F16, kind="ExternalInput").ap()
    nsrcd = nc.dram_tensor("nsrc", [128, 2 * NG], F16, kind="ExternalInput").ap()
    ndstd = nc.dram_tensor("ndst", [128, NBLK], F32, kind="ExternalInput").ap()
    wcatd = nc.dram_tensor("wcat", [nkch, 128, HOUT], F16, kind="ExternalInput").ap()
    bcat2d = nc.dram_tensor("bcat2", [2, HOUT], F16, kind="ExternalInput").ap()
    iotad = nc.dram_tensor("iota", [128, CG * 128], F16, kind="ExternalInput").ap()
    identd = nc.dram_tensor("ident", [128, 128], F32, kind="ExternalInput").ap()
    cntsd = nc.dram_tensor("cnts", [1, geom["ncalls"]], mybir.dt.int32, kind="ExternalInput").ap()
    out = nc.dram_tensor("out", [NSH, HOUT], F32, kind="ExternalOutput").ap()

    # call plan per (blk, half): list of group counts
    def calls_of(g):
        full, rem = divmod(g, CG)
        return [CG] * full + ([rem] if rem else [])

    calls_lo = calls_of(g_lo)
    calls_hi = calls_of(g_hi)

    with tile.TileContext(nc) as tc, ExitStack() as ctx:
        consts = ctx.enter_context(tc.tile_pool(name="consts", bufs=1))
        bigs = ctx.enter_context(tc.tile_pool(name="bigs", bufs=1))
        gatp = ctx.enter_context(tc.tile_pool(name="gat", bufs=GAT_BUFS))
        ohp = ctx.enter_context(tc.tile_pool(name="oh", bufs=6))
        sbB = ctx.enter_context(tc.tile_pool(name="sbB", bufs=2))
        psA = ctx.enter_context(tc.tile_pool(name="psA", bufs=2, space="PSUM"))
        psS = ctx.enter_context(tc.tile_pool(name="psS", bufs=2, space="PSUM"))
        psT = ctx.enter_context(tc.tile_pool(name="psT", bufs=2, space="PSUM"))
        psO = ctx.enter_context(tc.tile_pool(name="psO", bufs=2, space="PSUM"))

        # constants
        iota_sb = consts.tile([128, CG * 128], F16, tag="iota")
        nc.sync.dma_start(iota_sb[:], iotad[:])
        ident_sb = consts.tile([128, 128], F32, tag="ident")
        nc.sync.dma_start(ident_sb[:], identd[:])
        bcat2_sb = consts.tile([2, HOUT], F16, tag="bcat2")
        nc.sync.dma_start(bcat2_sb[:], bcat2d[:])
        ndst_sb = consts.tile([128, NBLK], F32, tag="ndst")
        nc.sync.dma_start(ndst_sb[:], ndstd[:])
        wcat_sb = []
        for k in range(nkch):
            w = consts.tile([128, HOUT], F16, tag=f"wcat{k}")
            nc.sync.dma_start(w[:], wcatd[k])
            wcat_sb.append(w)
        idx_sb = bigs.tile([128, S // 16], I16, tag="idx")
        nc.sync.dma_start(idx_sb[:], idx[:])
        dloc_sb = bigs.tile([128, NG], F16, tag="dloc")
        nc.sync.dma_start(dloc_sb[:], dlocd[:])
        nsrc_sb = bigs.tile([128, 2 * NG], F16, tag="nsrc")
        nc.sync.dma_start(nsrc_sb[:], nsrcd[:])
        cnts_sb = bigs.tile([1, geom["ncalls"]], mybir.dt.int32, tag="cnts")
        nc.sync.dma_start(cnts_sb[:], cntsd[:])

        hn_lo_view = hnd[0:min(SPLIT, N), :]
        hn_hi_view = hnd[SPLIT:N, :] if N > SPLIT else None
        qrr = [0]  # gather queue round-robin / call counter

        for bb in range(NBLK):
            rows = min(128, NSH - bb * 128)
            ps_main = psA.tile([128, IN], F32, tag="main")
            ps_s = psS.tile([2, 128], F32, tag="s")
            goff = bb * (g_lo + g_hi)  # group offset of this block
            n_emit = g_lo + (g_hi if hn_hi_view is not None else 0)
            gi = 0  # groups emitted so far for this block
            for half, (calls, base) in enumerate(
                [(calls_lo, hn_lo_view), (calls_hi, hn_hi_view)]
            ):
                if base is None:
                    continue
                g0 = goff + (g_lo if half else 0)
                coff = 0
                for ci, ng in enumerate(calls):
                    gcall = g0 + coff
                    icol0 = gcall * 8  # 128 idx / 16 per group
                    gat = gatp.tile([128, CG, IN], F16, tag="gat")
                    if qrr[0] < GAT_BUFS:
                        # First rotation of the pool: zero the physical
                        # buffers so skipped pad slots hold finite values
                        # (0 * garbage-NaN would poison the PSUM).
                        nc.vector.memset(gat[:], 0.0)
                    if PAD_SKIP:
                        nreg = nc.alloc_registers(engines=(mybir.EngineType.Pool,))
                        nc.gpsimd.reg_load(nreg, cnts_sb[0:1, qrr[0] : qrr[0] + 1])
                        cnt = nreg
                    else:
                        cnt = ng * 128
                    nc.gpsimd.dma_gather(
                        gat[:, :ng, :],
                        base,
                        idx_sb[:, icol0 : icol0 + ng * 8],
                        ng * 128,
                        cnt,
                        IN,
                        queue_num=qrr[0] % 4,
                    )
                    qrr[0] += 1
                    oh = ohp.tile([128, CG * 128], F16, tag="oh")
                    dl = (
                        dloc_sb[:, gcall : gcall + ng]
                        .unsqueeze(2)
                        .broadcast_to([128, ng, 128])
                    )
                    nc.vector.tensor_tensor(
                        oh[:, : ng * 128],
                        iota_sb[:, : ng * 128],
                        dl,
                        mybir.AluOpType.is_equal,
                    )
                    for g in range(ng):
                        lhs = oh[:, g * 128 : (g + 1) * 128]
                        gg = gcall + g
                        first = gi == 0
                        last = gi == n_emit - 1
                        nc.tensor.matmul(
                            ps_main[:], lhs, gat[:, g, :], start=first, stop=last
                        )
                        nc.tensor.matmul(
                            ps_s[:],
                            nsrc_sb[:, 2 * gg : 2 * gg + 2],
                            lhs,
                            start=first,
                            stop=last,
                        )
                        gi += 1
                    coff += ng
            assert gi == n_emit

            # phase B (fp32 transpose, fp16 Wcat streams: 1 cyc/row on PE)
            aggH = sbB.tile([128, IN], F32, tag="aggH")
            nc.scalar.copy(aggH[:], ps_main[:])
            s_sb = sbB.tile([2, 128], F16, tag="s_sb")
            nc.vector.tensor_copy(s_sb[:], ps_s[:])
            aggHT = sbB.tile([128, IN], F16, tag="aggHT")
            for k in range(nkch):
                ps_t = psT.tile([128, 128], F32, tag="pt")
                nc.tensor.transpose(ps_t[:], aggH[:, k * 128 : (k + 1) * 128], ident_sb[:])
                nc.vector.tensor_copy(aggHT[:, k * 128 : (k + 1) * 128], ps_t[:])
            ps_o = psO.tile([128, HOUT], F32, tag="po")
            for k in range(nkch):
                nc.tensor.matmul(
                    ps_o[:],
                    aggHT[:, k * 128 : (k + 1) * 128],
                    wcat_sb[k][:],
                    start=(k == 0),
                    stop=False,
                )
            nc.tensor.matmul(ps_o[:], s_sb[:], bcat2_sb[:], start=False, stop=True)
            outsb = sbB.tile([128, HOUT], F32, tag="outsb")
            nc.scalar.activation(
                outsb[:],
                ps_o[:],
                mybir.ActivationFunctionType.Relu,
                scale=ndst_sb[:, bb : bb + 1],
            )
            nc.sync.dma_start(out[bb * 128 : bb * 128 + rows, :], outsb[:rows, :])

    nc.compile()
    return nc


_CACHE = {}


def kernel(h, W, b, norm, src, dst):
    h = np.asarray(h)
    in_maps, geom, shard_rows = _prep(h, W, b, norm, src, dst)
    key = tuple(sorted(geom.items()))
    if key not in _CACHE:
        _CACHE[key] = _build(geom)
    nc = _CACHE[key]
    res = run_bass_kernel_spmd(
        nc, in_maps, list(range(NCORES)), trace=TRACE
    )
    shards = [
        res.results[c]["out"][shard_rows[c]] for c in range(NCORES)
    ]
    out = np.concatenate(shards, axis=0).astype(np.float32)
    if TRACE and res.exec_time_ns is not None:
        print(f"HW exec time: {res.exec_time_ns} ns")
    kernel._last = res
    return out
